# revision 8
# baseline (speedup 1.0000x reference)
"""Trainium2 Bass kernel for nn_DeformLikeASPPConv (8-core data parallel), v2.

Per-core pipeline (one sample [64, 256, 256] per NeuronCore):
  Phase A: offset head (3x3 conv, 2 out ch) via 18-tap matmul + block-diagonal
           select matmuls over a 7-row-slot staging tile (fp16, no shift DMAs).
  Maps:    coordinate chain + 4 bilinear corner-weight products + relative
           int16 gather indices, all in compact [128, 512] layout.
  Phase B: dma_gather of 4 corners (fp16), 4-weight blend on DVE, cross-half
           reduce, writes warped image W2 (fp16) with dual-row copy.
  Phase C: dilated 3x3 conv as 6 accumulating matmuls/row (2-row PSUM chunks)
           + BN + ReLU, interleaved with Phase B blocks for engine overlap.
"""
import sys
if "/opt/trn_rl_repo" not in sys.path:
    sys.path.insert(0, "/opt/trn_rl_repo")
import numpy as np
import concourse.bass as bass
import concourse.bacc as bacc
import concourse.tile as tile
import concourse.mybir as mybir
from concourse import bass_utils

N_CORES = 8
H, W = 256, 256
C = 64
DIL = 12
BN_EPS = 1e-5
N = H * W
CF = N // 128          # 512, compact layout cols
ADV = 10               # Phase A rows per sub-block
SUP = 20               # Phase A rows per super-block
MB = 8 * W             # Phase B/C pixels per block (8 rows)
NBLK = H // 8          # 32

F32 = mybir.dt.float32
F16 = mybir.dt.float16
I16 = mybir.dt.int16
I32 = mybir.dt.int32
ALU = mybir.AluOpType
AF = mybir.ActivationFunctionType


def prep_shared(offset_w, offset_b, conv_w, bn_gamma, bn_beta, bn_mean,
                bn_var):
    """Sample-independent inputs."""
    wo18 = np.zeros((C, 32), np.float32)
    for t in range(9):
        r, s = t // 3, t % 3
        for o in range(2):
            wo18[:, 2 * t + o] = offset_w[o, :, r, s]
    sel = {}
    for T in (0, 1):
        for h in (0, 1):
            for ds in (-1, 0, 1):
                L = np.zeros((96, 2 * ADV), np.float32)
                for s in range(3):
                    gs = s + 3 * T
                    for t in range(9):
                        dr, sds = t // 3 - 1, t % 3 - 1
                        if sds != ds:
                            continue
                        qp = 2 * gs + h - 1 - dr
                        if not (0 <= qp < ADV):
                            continue
                        for o in range(2):
                            L[32 * s + 2 * t + o, o * ADV + qp] = 1.0
                sel[(T, h, ds)] = L
    offb10 = np.repeat(offset_b.astype(np.float32), ADV).reshape(2 * ADV, 1)
    inv = (bn_gamma / np.sqrt(bn_var + BN_EPS)).astype(np.float32)
    wmf = conv_w * inv[:, None, None, None]
    wm1 = np.zeros((C, 3 * C), np.float32)
    wm1a = np.zeros((C, 3 * C), np.float32)
    wm1b = np.zeros((C, 3 * C), np.float32)
    wm2 = np.zeros((2 * C, 3 * C), np.float32)
    for gs in range(3):
        wm1[:, gs * C:(gs + 1) * C] = wmf[:, :, 1, gs].T
        wm1a[:, gs * C:(gs + 1) * C] = wmf[:, :, 0, gs].T
        wm1b[:, gs * C:(gs + 1) * C] = wmf[:, :, 2, gs].T
        wm2[0:C, gs * C:(gs + 1) * C] = wmf[:, :, 0, gs].T
        wm2[C:2 * C, gs * C:(gs + 1) * C] = wmf[:, :, 2, gs].T
    biasy = (bn_beta - bn_mean * inv).astype(np.float32).reshape(C, 1)
    pix = np.arange(N).reshape(128, CF)
    jmap = (pix % W).astype(np.float32)
    imap = (pix // W).astype(np.float32)
    rows = pix // W
    basemap = (W * np.maximum(0, 8 * (rows // 8) - 2)).astype(np.float32)
    return {
        "wo18": wo18.astype(np.float16),
        **{f"sel_{T}_{h}_{ds + 1}": sel[(T, h, ds)].astype(np.float16)
           for T in (0, 1) for h in (0, 1) for ds in (-1, 0, 1)},
        "offb10": offb10,
        "wm1": wm1.astype(np.float16),
        "wm1a": wm1a.astype(np.float16),
        "wm1b": wm1b.astype(np.float16),
        "wm2": wm2.astype(np.float16),
        "biasy": biasy,
        "jmap": jmap,
        "imap": imap,
        "basemap": basemap,
    }


def prep_sample(x):
    """x: [C, H, W] fp32 one sample."""
    x_cm16 = x.reshape(C, N).astype(np.float16)
    pm = np.ascontiguousarray(x.reshape(C, N).T).astype(np.float16)
    p = np.arange(N)
    x_pm16 = np.concatenate(
        [pm[np.minimum(p + d, N - 1)] for d in (0, 1, W, W + 1)], axis=1)
    return {"x_cm16": x_cm16, "x_pm16": np.ascontiguousarray(x_pm16)}


IN_SPECS = [
    ("x_cm16", (C, N), np.float16),
    ("x_pm16", (N, 4 * C), np.float16),
    ("wo18", (C, 32), np.float16),
] + [(f"sel_{T}_{h}_{d}", (96, 2 * ADV), np.float16)
     for T in (0, 1) for h in (0, 1) for d in (0, 1, 2)] + [
    ("offb10", (2 * ADV, 1), np.float32),
    ("wm1", (C, 3 * C), np.float16),
    ("wm1a", (C, 3 * C), np.float16),
    ("wm1b", (C, 3 * C), np.float16),
    ("wm2", (2 * C, 3 * C), np.float16),
    ("biasy", (C, 1), np.float32),
    ("jmap", (128, CF), np.float32),
    ("imap", (128, CF), np.float32),
    ("basemap", (128, CF), np.float32),
]


def emit(tc, io):
    nc = tc.nc
    CLX = (W - 2) + 0.99609375
    CLY = (H - 2) + 0.99609375

    x_cm16, x_pm16 = io["x_cm16"], io["x_pm16"]
    y_out = io["y"]

    with tc.tile_pool(name="dram", bufs=1, space="DRAM") as dramp, \
         tc.tile_pool(name="consts", bufs=1) as cstp:
        ox_dram = dramp.tile([2, N + 4 * W], F32)
        idx16d = dramp.tile([1, N], I16)
        idx16w = [dramp.tile([128, N // 64], I16, name=f"idx16w{c}")
                  for c in range(4)]
        w4d = dramp.tile([4, N], F16)

        wo18_s = cstp.tile([C, 32], F16, tag="wo18")
        nc.sync.dma_start(wo18_s[:], io["wo18"][:])
        sel_s = {}
        for T in (0, 1):
            for h in (0, 1):
                for d in (0, 1, 2):
                    nm = f"sel_{T}_{h}_{d}"
                    sel_s[nm] = cstp.tile([96, 2 * ADV], F16, tag=nm, name=nm)
                    nc.sync.dma_start(sel_s[nm][:], io[nm][:])
        offb10_s = cstp.tile([2 * ADV, 1], F32, tag="offb10")
        nc.sync.dma_start(offb10_s[:], io["offb10"][:])
        biasy_s = cstp.tile([C, 1], F32, tag="biasy")
        nc.sync.dma_start(biasy_s[:], io["biasy"][:])

        # ---------------- Phase A: offset head ----------------
        with tc.tile_pool(name="xa", bufs=2) as xap, \
             tc.tile_pool(name="sA", bufs=3) as sap, \
             tc.tile_pool(name="oxs", bufs=2) as oxp, \
             tc.tile_pool(name="psA", bufs=2, space="PSUM") as psA, \
             tc.tile_pool(name="psA2", bufs=2, space="PSUM") as psA2:
            PHS = W + 1  # half stride in staging: [b|rowA|b|rowB|b]
            pend = None

            def emit_select(sAt, r0):
                ps2 = psA2.tile([2 * ADV, W], F32, tag="psA2")
                k = 0
                for T in (0, 1):
                    for h in (0, 1):
                        for d in (0, 1, 2):
                            nc.tensor.matmul(
                                ps2[:], sel_s[f"sel_{T}_{h}_{d}"][:],
                                sAt[T][:, h * PHS + d:h * PHS + d + W],
                                start=(k == 0), stop=(k == 11))
                            k += 1
                oxs = oxp.tile([2 * ADV, W], F32, tag="oxs")
                nc.scalar.activation(oxs[:], ps2[:], AF.Tanh,
                                     bias=offb10_s[:], scale=1.0)
                nc.sync.dma_start(
                    bass.AP(tensor=ox_dram[:].tensor,
                            offset=ox_dram[:].offset + r0 * W,
                            ap=[[N + 4 * W, 2], [W, ADV], [1, W]]),
                    oxs[:])

            for r0s in range(0, H, SUP):
                nsup = min(SUP, H - r0s)
                lo = max(0, r0s - 1)
                hi = min(H, r0s + nsup + 1)
                xa = xap.tile([C, (SUP + 2) * W], F16, tag="xa")
                nc.sync.dma_start(xa[:, 0:(hi - lo) * W],
                                  x_cm16[:, lo * W:hi * W])
                for r0 in range(r0s, r0s + nsup, ADV):
                    nr = min(ADV, H - r0)
                    pst = [psA.tile([96, 2 * W], F32, tag=f"psa{T}",
                                    name=f"psa{T}") for T in (0, 1)]
                    sAt = [sap.tile([96, 2 * W + 3], F16, tag=f"sAA{T}",
                                    name=f"sA{T}") for T in (0, 1)]
                    # per-slot pair matmuls [32, 2W]
                    pair_ok = []
                    for gs in range(6):
                        T, s = gs // 3, gs % 3
                        ra = r0 - 1 + 2 * gs
                        # full in-range pair -> one [32, 2W] matmul
                        if 0 <= ra and ra + 1 < H:
                            nc.tensor.matmul(
                                pst[T][32 * s:32 * s + 32, :], wo18_s[:],
                                xa[:, (ra - lo) * W:(ra - lo + 2) * W],
                                start=True, stop=True)
                            pair_ok.append(gs)
                        else:
                            for hh in (0, 1):
                                rr = ra + hh
                                if 0 <= rr < H:
                                    nc.tensor.matmul(
                                        pst[T][32 * s:32 * s + 32,
                                               hh * W:hh * W + W],
                                        wo18_s[:],
                                        xa[:, (rr - lo) * W:(rr - lo + 1) * W],
                                        start=True, stop=True)
                    for T in (0, 1):
                        t = sAt[T]
                        nc.gpsimd.memset(t[:, 0:1], 0.0)
                        nc.gpsimd.memset(t[:, PHS:PHS + 1], 0.0)
                        nc.gpsimd.memset(t[:, 2 * PHS:2 * PHS + 1], 0.0)
                        # copy both halves out of PSUM
                        nc.vector.tensor_copy(t[:, 1:W + 1], pst[T][:, 0:W])
                        nc.vector.tensor_copy(t[:, PHS + 1:PHS + 1 + W],
                                              pst[T][:, W:2 * W])
                        # zero out-of-image rows
                        for gs in range(3 * T, 3 * T + 3):
                            s = gs % 3
                            for hh in (0, 1):
                                rr = r0 - 1 + 2 * gs + hh
                                if not (0 <= rr < H):
                                    nc.vector.memset(
                                        t[32 * s:32 * s + 32,
                                          hh * PHS + 1:hh * PHS + 1 + W], 0.0)
                    if pend is not None:
                        emit_select(*pend)
                    pend = (sAt, r0)
            emit_select(*pend)

        # ---------------- Maps (compact [128, CF]) ----------------
        with tc.tile_pool(name="mp", bufs=1) as mp:
            jm = mp.tile([128, CF], F32, tag="jm")
            nc.sync.dma_start(jm[:], io["jmap"][:])
            im = mp.tile([128, CF], F32, tag="im")
            nc.sync.dma_start(im[:], io["imap"][:])
            bm = mp.tile([128, CF], F32, tag="bm")
            nc.sync.dma_start(bm[:], io["basemap"][:])

            def coord_chain(row, base_map, clmax):
                oc = mp.tile([128, CF], F32, tag=f"oc{row}")
                nc.sync.dma_start(
                    oc[:], bass.AP(tensor=ox_dram[:].tensor,
                                   offset=ox_dram[:].offset
                                   + row * (N + 4 * W),
                                   ap=[[CF, 128], [1, CF]]))
                ic = mp.tile([128, CF], F32, tag=f"ic{row}")
                nc.vector.scalar_tensor_tensor(ic[:], oc[:], 2.0, base_map[:],
                                               ALU.mult, ALU.add)
                nc.vector.tensor_scalar(ic[:], ic[:], 0.0, clmax,
                                        ALU.max, ALU.min)
                i32 = mp.tile([128, CF], I32, tag=f"i32{row}")
                nc.vector.tensor_copy(i32[:], ic[:])
                c0f = mp.tile([128, CF], F32, tag=f"c0f{row}")
                nc.vector.tensor_copy(c0f[:], i32[:])
                wf = mp.tile([128, CF], F32, tag=f"wf{row}")
                nc.vector.tensor_tensor(wf[:], ic[:], c0f[:], ALU.subtract)
                msk = mp.tile([128, CF], F32, tag=f"msk{row}")
                nc.vector.tensor_scalar(msk[:], wf[:], 0.0, None, ALU.is_lt)
                nc.vector.tensor_tensor(c0f[:], c0f[:], msk[:], ALU.subtract)
                nc.vector.tensor_tensor(wf[:], ic[:], c0f[:], ALU.subtract)
                # 1 - w on the scalar engine (scale=-1, bias=+1)
                w1m = mp.tile([128, CF], F32, tag=f"w1m{row}")
                nc.scalar.activation(w1m[:], wf[:], AF.Copy, bias=0.0,
                                     scale=-1.0)
                nc.vector.tensor_scalar(w1m[:], w1m[:], 1.0, None, ALU.add)
                return c0f, wf, w1m

            x0f, wx, wx1m = coord_chain(0, jm, CLX)
            y0f, wy, wy1m = coord_chain(1, im, CLY)
            for row, (a, b) in enumerate(
                    ((wx1m, wy1m), (wx, wy1m), (wx1m, wy), (wx, wy))):
                wprod = mp.tile([128, CF], F16, tag=f"wp{row}")
                nc.vector.tensor_tensor(wprod[:], a[:], b[:], ALU.mult)
                nc.sync.dma_start(w4d[row:row + 1, :], wprod[:])
            idxf = mp.tile([128, CF], F32, tag="idxf")
            nc.vector.scalar_tensor_tensor(idxf[:], y0f[:], float(W), x0f[:],
                                           ALU.mult, ALU.add)
            nc.vector.tensor_tensor(idxf[:], idxf[:], bm[:], ALU.subtract)
            idx16 = mp.tile([128, CF], I16, tag="idx16")
            nc.vector.tensor_copy(idx16[:], idxf[:])
            nc.sync.dma_start(idx16d[:], idx16[:])


        # ---------------- Phase B || Phase C ----------------
        with tc.tile_pool(name="w2", bufs=1) as w2p, \
             tc.tile_pool(name="wc", bufs=1) as wc:
            W2 = w2p.tile([128, N + 2 * W], F16, tag="W2")
            wm1_s = wc.tile([C, 3 * C], F16, tag="wm1")
            nc.sync.dma_start(wm1_s[:], io["wm1"][:])
            wm1a_s = wc.tile([C, 3 * C], F16, tag="wm1a")
            nc.sync.dma_start(wm1a_s[:], io["wm1a"][:])
            wm1b_s = wc.tile([C, 3 * C], F16, tag="wm1b")
            nc.sync.dma_start(wm1b_s[:], io["wm1b"][:])
            wm2_s = wc.tile([2 * C, 3 * C], F16, tag="wm2")
            nc.sync.dma_start(wm2_s[:], io["wm2"][:])

            with tc.tile_pool(name="gb", bufs=2) as gbp, \
                 tc.tile_pool(name="wtb", bufs=2) as wtp, \
                 tc.tile_pool(name="ixb", bufs=2) as ixp, \
                 tc.tile_pool(name="plb", bufs=2) as plp, \
                 tc.tile_pool(name="yb", bufs=3) as ybp, \
                 tc.tile_pool(name="psC", bufs=4, space="PSUM") as psC:
                CHK = N // 64  # wrapped cols per chunk tile (8 blocks each)

                def emit_wrap_chunk(c):
                    idxw_s = ixp.tile([16, CHK], I16, tag=f"idxw{c % 2}",
                                      name=f"idxw{c}")
                    nc.sync.dma_start(
                        idxw_s[:],
                        bass.AP(tensor=idx16d[:].tensor,
                                offset=idx16d[:].offset + c * CHK * 16,
                                ap=[[1, 16], [16, CHK]]))
                    for rep in range(8):
                        nc.sync.dma_start(idx16w[c][16 * rep:16 * rep + 16, :],
                                          idxw_s[:])

                emit_wrap_chunk(0)
                for k in range(NBLK + 2):
                    if k < NBLK:
                        if k % 8 == 0 and k // 8 + 1 < 4:
                            emit_wrap_chunk(k // 8 + 1)
                        # ---- Phase B block k: rows [8k, 8k+8) ----
                        r0 = 8 * k
                        base_px = max(0, r0 - 2) * W
                        idxb = ixp.tile([128, MB // 16], I16, tag="idxb")
                        iwt = idx16w[k // 8]
                        nc.sync.dma_start(
                            idxb[:],
                            bass.AP(tensor=iwt[:].tensor,
                                    offset=iwt[:].offset
                                    + (k % 8) * (MB // 16),
                                    ap=[[N // 64, 128], [1, MB // 16]]))
                        g = gbp.tile([128, 2, MB], F16, tag="g")
                        nc.gpsimd.dma_gather(
                            g[:],
                            bass.AP(tensor=x_pm16[:].tensor,
                                    offset=x_pm16[:].offset + base_px * 4 * C,
                                    ap=[[4 * C, N - base_px], [1, 4 * C]]),
                            idxb[:], MB, MB, 4 * C, transpose=True,
                            single_packet=False)
                        w0t = wtp.tile([128, MB], F16, tag="w0t")
                        nc.sync.dma_start(
                            w0t[:], bass.AP(tensor=w4d[:].tensor,
                                            offset=w4d[:].offset + r0 * W,
                                            ap=[[N, 2], [0, C], [1, MB]]))
                        w1t = wtp.tile([128, MB], F16, tag="w1t")
                        nc.sync.dma_start(
                            w1t[:], bass.AP(tensor=w4d[:].tensor,
                                            offset=w4d[:].offset
                                            + 2 * N + r0 * W,
                                            ap=[[N, 2], [0, C], [1, MB]]))
                        g0 = g[:, 0, :]
                        g1 = g[:, 1, :]
                        nc.vector.tensor_tensor(g0, g0, w0t[:], ALU.mult)
                        nc.vector.tensor_tensor(g1, g1, w1t[:], ALU.mult)
                        nc.vector.tensor_tensor(g0, g0, g1, ALU.add)
                        pl = plp.tile([C, MB], F16, tag="pl")
                        nc.scalar.copy(pl[:], g[C:2 * C, 0, :])
                        nc.vector.tensor_tensor(
                            W2[0:C, r0 * W:(r0 + 8) * W], g[0:C, 0, :],
                            pl[:], ALU.add)
                        blo = max(2 * DIL, r0)
                        if blo < r0 + 8:
                            nc.vector.tensor_copy(
                                W2[C:2 * C,
                                   (blo - 2 * DIL) * W:(r0 + 8 - 2 * DIL) * W],
                                W2[0:C, blo * W:(r0 + 8) * W])
                    if k >= 2:
                        # ---- Phase C block k-2: rows [8(k-2), 8(k-2)+8) ----
                        rc0 = 8 * (k - 2)
                        yb = ybp.tile([C, 8 * W], F16, tag="yb")
                        for rr0 in range(rc0, rc0 + 8, 2):
                            ps = psC.tile([C, 2 * W], F32, tag="psC")
                            mms = []
                            for ri in range(2):
                                r = rr0 + ri
                                seg = {-1: (DIL, W, -DIL), 0: (0, W, 0),
                                       1: (0, W - DIL, DIL)}
                                for ds in (0, -1, 1):
                                    olo, ohi, dsoff = seg[ds]
                                    gcol = (ds + 1) * C
                                    pv = ps[:, ri * W + olo:ri * W + ohi]
                                    mms.append(
                                        (pv, wm1_s[:, gcol:gcol + C],
                                         W2[0:C, r * W + olo + dsoff:
                                            r * W + ohi + dsoff]))
                                    if DIL <= r < H - DIL:
                                        mms.append(
                                            (pv, wm2_s[:, gcol:gcol + C],
                                             W2[:, (r - DIL) * W + olo + dsoff:
                                                (r - DIL) * W + ohi + dsoff]))
                                    elif r < DIL:
                                        mms.append(
                                            (pv, wm1b_s[:, gcol:gcol + C],
                                             W2[0:C,
                                                (r + DIL) * W + olo + dsoff:
                                                (r + DIL) * W + ohi + dsoff]))
                                    else:
                                        mms.append(
                                            (pv, wm1a_s[:, gcol:gcol + C],
                                             W2[0:C,
                                                (r - DIL) * W + olo + dsoff:
                                                (r - DIL) * W + ohi + dsoff]))
                            for i, (o, l, rr) in enumerate(mms):
                                nc.tensor.matmul(o, l, rr, start=(i == 0),
                                                 stop=(i == len(mms) - 1))
                            nc.scalar.activation(
                                yb[:, (rr0 - rc0) * W:(rr0 - rc0 + 2) * W],
                                ps[:], AF.Relu, bias=biasy_s[:], scale=1.0)
                        nc.sync.dma_start(y_out[:, rc0 * W:(rc0 + 8) * W],
                                          yb[:])


def build_module(num_devices):
    nc = bacc.Bacc("TRN2", target_bir_lowering=False, debug=False,
                   num_devices=num_devices)
    io = {}
    for name, shape, dt in IN_SPECS:
        mdt = {np.float16: F16, np.float32: F32}[dt]
        io[name] = nc.dram_tensor(name, list(shape), mdt,
                                  kind="ExternalInput").ap()
    io["y"] = nc.dram_tensor("y", [C, N], F16, kind="ExternalOutput").ap()
    with tile.TileContext(nc) as tc:
        emit(tc, io)
    nc.compile()
    return nc


_NC_CACHE = {}


def kernel(x, offset_w, offset_b, conv_w, bn_gamma, bn_beta, bn_mean, bn_var):
    x = np.asarray(x, np.float32)
    offset_w = np.asarray(offset_w, np.float32)
    offset_b = np.asarray(offset_b, np.float32)
    conv_w = np.asarray(conv_w, np.float32)
    bn_gamma = np.asarray(bn_gamma, np.float32)
    bn_beta = np.asarray(bn_beta, np.float32)
    bn_mean = np.asarray(bn_mean, np.float32)
    bn_var = np.asarray(bn_var, np.float32)
    B = x.shape[0]
    if "nc" not in _NC_CACHE:
        _NC_CACHE["nc"] = build_module(N_CORES)
    nc = _NC_CACHE["nc"]
    shared = prep_shared(offset_w, offset_b, conv_w, bn_gamma, bn_beta,
                         bn_mean, bn_var)
    in_maps = []
    for b in range(B):
        m = dict(shared)
        m.update(prep_sample(x[b]))
        in_maps.append(m)
    res = bass_utils.run_bass_kernel_spmd(nc, in_maps,
                                          core_ids=list(range(B)))
    out = np.stack([res.results[b]["y"].reshape(C, H, W) for b in range(B)])
    return out.astype(np.float32)


# revision 9
# speedup vs baseline: 1.0182x; 1.0182x over previous
"""Trainium2 Bass kernel for nn_DeformLikeASPPConv (8-core data parallel), v2.

Per-core pipeline (one sample [64, 256, 256] per NeuronCore):
  Phase A: offset head (3x3 conv, 2 out ch) via 18-tap matmul + block-diagonal
           select matmuls over a 7-row-slot staging tile (fp16, no shift DMAs).
  Maps:    coordinate chain + 4 bilinear corner-weight products + relative
           int16 gather indices, all in compact [128, 512] layout.
  Phase B: dma_gather of 4 corners (fp16), 4-weight blend on DVE, cross-half
           reduce, writes warped image W2 (fp16) with dual-row copy.
  Phase C: dilated 3x3 conv as 6 accumulating matmuls/row (2-row PSUM chunks)
           + BN + ReLU, interleaved with Phase B blocks for engine overlap.
"""
import sys
if "/opt/trn_rl_repo" not in sys.path:
    sys.path.insert(0, "/opt/trn_rl_repo")
import numpy as np
import concourse.bass as bass
import concourse.bacc as bacc
import concourse.tile as tile
import concourse.mybir as mybir
from concourse import bass_utils

N_CORES = 8
H, W = 256, 256
C = 64
DIL = 12
BN_EPS = 1e-5
N = H * W
CF = N // 128          # 512, compact layout cols
ADV = 10               # Phase A rows per sub-block
SUP = 20               # Phase A rows per super-block
MB = 8 * W             # Phase B/C pixels per block (8 rows)
NBLK = H // 8          # 32

F32 = mybir.dt.float32
F16 = mybir.dt.float16
I16 = mybir.dt.int16
I32 = mybir.dt.int32
ALU = mybir.AluOpType
AF = mybir.ActivationFunctionType


def prep_shared(offset_w, offset_b, conv_w, bn_gamma, bn_beta, bn_mean,
                bn_var):
    """Sample-independent inputs."""
    wo18 = np.zeros((C, 32), np.float32)
    for t in range(9):
        r, s = t // 3, t % 3
        for o in range(2):
            wo18[:, 2 * t + o] = offset_w[o, :, r, s]
    sel = {}
    for T in (0, 1):
        for h in (0, 1):
            for ds in (-1, 0, 1):
                L = np.zeros((96, 2 * ADV), np.float32)
                for s in range(3):
                    gs = s + 3 * T
                    for t in range(9):
                        dr, sds = t // 3 - 1, t % 3 - 1
                        if sds != ds:
                            continue
                        qp = 2 * gs + h - 1 - dr
                        if not (0 <= qp < ADV):
                            continue
                        for o in range(2):
                            L[32 * s + 2 * t + o, o * ADV + qp] = 1.0
                sel[(T, h, ds)] = L
    offb10 = np.repeat(offset_b.astype(np.float32), ADV).reshape(2 * ADV, 1)
    inv = (bn_gamma / np.sqrt(bn_var + BN_EPS)).astype(np.float32)
    wmf = conv_w * inv[:, None, None, None]
    wm1 = np.zeros((C, 3 * C), np.float32)
    wm1a = np.zeros((C, 3 * C), np.float32)
    wm1b = np.zeros((C, 3 * C), np.float32)
    wm2 = np.zeros((2 * C, 3 * C), np.float32)
    for gs in range(3):
        wm1[:, gs * C:(gs + 1) * C] = wmf[:, :, 1, gs].T
        wm1a[:, gs * C:(gs + 1) * C] = wmf[:, :, 0, gs].T
        wm1b[:, gs * C:(gs + 1) * C] = wmf[:, :, 2, gs].T
        wm2[0:C, gs * C:(gs + 1) * C] = wmf[:, :, 0, gs].T
        wm2[C:2 * C, gs * C:(gs + 1) * C] = wmf[:, :, 2, gs].T
    biasy = (bn_beta - bn_mean * inv).astype(np.float32).reshape(C, 1)
    pix = np.arange(N).reshape(128, CF)
    jmap = (pix % W).astype(np.float32)
    imap = (pix // W).astype(np.float32)
    rows = pix // W
    basemap = (W * np.maximum(0, 8 * (rows // 8) - 2)).astype(np.float32)
    return {
        "wo18": wo18.astype(np.float16),
        **{f"sel_{T}_{h}_{ds + 1}": sel[(T, h, ds)].astype(np.float16)
           for T in (0, 1) for h in (0, 1) for ds in (-1, 0, 1)},
        "offb10": offb10,
        "wm1": wm1.astype(np.float16),
        "wm1a": wm1a.astype(np.float16),
        "wm1b": wm1b.astype(np.float16),
        "wm2": wm2.astype(np.float16),
        "biasy": biasy,
        "jmap": jmap,
        "imap": imap,
        "basemap": basemap,
    }


def prep_sample(x):
    """x: [C, H, W] fp32 one sample."""
    x_cm16 = x.reshape(C, N).astype(np.float16)
    pm = np.ascontiguousarray(x.reshape(C, N).T).astype(np.float16)
    p = np.arange(N)
    x_pm16 = np.concatenate(
        [pm[np.minimum(p + d, N - 1)] for d in (0, 1, W, W + 1)], axis=1)
    return {"x_cm16": x_cm16, "x_pm16": np.ascontiguousarray(x_pm16)}


IN_SPECS = [
    ("x_cm16", (C, N), np.float16),
    ("x_pm16", (N, 4 * C), np.float16),
    ("wo18", (C, 32), np.float16),
] + [(f"sel_{T}_{h}_{d}", (96, 2 * ADV), np.float16)
     for T in (0, 1) for h in (0, 1) for d in (0, 1, 2)] + [
    ("offb10", (2 * ADV, 1), np.float32),
    ("wm1", (C, 3 * C), np.float16),
    ("wm1a", (C, 3 * C), np.float16),
    ("wm1b", (C, 3 * C), np.float16),
    ("wm2", (2 * C, 3 * C), np.float16),
    ("biasy", (C, 1), np.float32),
    ("jmap", (128, CF), np.float32),
    ("imap", (128, CF), np.float32),
    ("basemap", (128, CF), np.float32),
]


def emit(tc, io):
    nc = tc.nc
    CLX = (W - 2) + 0.99609375
    CLY = (H - 2) + 0.99609375

    x_cm16, x_pm16 = io["x_cm16"], io["x_pm16"]
    y_out = io["y"]

    with tc.tile_pool(name="dram", bufs=1, space="DRAM") as dramp, \
         tc.tile_pool(name="consts", bufs=1) as cstp:
        ox_dram = dramp.tile([2, N + 4 * W], F32)
        idx16d = dramp.tile([1, N], I16)
        idx16w = [dramp.tile([128, N // 64], I16, name=f"idx16w{c}")
                  for c in range(4)]
        w4d = dramp.tile([4, N], F16)

        wo18_s = cstp.tile([C, 32], F16, tag="wo18")
        nc.sync.dma_start(wo18_s[:], io["wo18"][:])
        sel_s = {}
        for T in (0, 1):
            for h in (0, 1):
                for d in (0, 1, 2):
                    nm = f"sel_{T}_{h}_{d}"
                    sel_s[nm] = cstp.tile([96, 2 * ADV], F16, tag=nm, name=nm)
                    nc.sync.dma_start(sel_s[nm][:], io[nm][:])
        offb10_s = cstp.tile([2 * ADV, 1], F32, tag="offb10")
        nc.sync.dma_start(offb10_s[:], io["offb10"][:])
        biasy_s = cstp.tile([C, 1], F32, tag="biasy")
        nc.sync.dma_start(biasy_s[:], io["biasy"][:])

        # ---------------- Phase A: offset head ----------------
        with tc.tile_pool(name="xa", bufs=2) as xap, \
             tc.tile_pool(name="sA", bufs=3) as sap, \
             tc.tile_pool(name="oxs", bufs=2) as oxp, \
             tc.tile_pool(name="psA", bufs=2, space="PSUM") as psA, \
             tc.tile_pool(name="psA2", bufs=2, space="PSUM") as psA2:
            PHS = W + 1  # half stride in staging: [b|rowA|b|rowB|b]

            def emit_select(sAt, r0):
                ps2 = psA2.tile([2 * ADV, W], F32, tag="psA2")
                k = 0
                for T in (0, 1):
                    for h in (0, 1):
                        for d in (0, 1, 2):
                            nc.tensor.matmul(
                                ps2[:], sel_s[f"sel_{T}_{h}_{d}"][:],
                                sAt[T][:, h * PHS + d:h * PHS + d + W],
                                start=(k == 0), stop=(k == 11))
                            k += 1
                oxs = oxp.tile([2 * ADV, W], F32, tag="oxs")
                nc.scalar.activation(oxs[:], ps2[:], AF.Tanh,
                                     bias=offb10_s[:], scale=1.0)
                nc.sync.dma_start(
                    bass.AP(tensor=ox_dram[:].tensor,
                            offset=ox_dram[:].offset + r0 * W,
                            ap=[[N + 4 * W, 2], [W, ADV], [1, W]]),
                    oxs[:])

            for r0s in range(0, H, SUP):
                nsup = min(SUP, H - r0s)
                lo = max(0, r0s - 1)
                hi = min(H, r0s + nsup + 1)
                xa = xap.tile([C, (SUP + 2) * W], F16, tag="xa")
                nc.sync.dma_start(xa[:, 0:(hi - lo) * W],
                                  x_cm16[:, lo * W:hi * W])
                for r0 in range(r0s, r0s + nsup, ADV):
                    nr = min(ADV, H - r0)
                    pst = [psA.tile([96, 2 * W], F32, tag=f"psa{T}",
                                    name=f"psa{T}") for T in (0, 1)]
                    sAt = [sap.tile([96, 2 * W + 3], F16, tag=f"sAA{T}",
                                    name=f"sA{T}") for T in (0, 1)]
                    # per-slot pair matmuls [32, 2W]
                    pair_ok = []
                    for gs in range(6):
                        T, s = gs // 3, gs % 3
                        ra = r0 - 1 + 2 * gs
                        # full in-range pair -> one [32, 2W] matmul
                        if 0 <= ra and ra + 1 < H:
                            nc.tensor.matmul(
                                pst[T][32 * s:32 * s + 32, :], wo18_s[:],
                                xa[:, (ra - lo) * W:(ra - lo + 2) * W],
                                start=True, stop=True)
                            pair_ok.append(gs)
                        else:
                            for hh in (0, 1):
                                rr = ra + hh
                                if 0 <= rr < H:
                                    nc.tensor.matmul(
                                        pst[T][32 * s:32 * s + 32,
                                               hh * W:hh * W + W],
                                        wo18_s[:],
                                        xa[:, (rr - lo) * W:(rr - lo + 1) * W],
                                        start=True, stop=True)
                    for T in (0, 1):
                        t = sAt[T]
                        nc.gpsimd.memset(t[:, 0:1], 0.0)
                        nc.gpsimd.memset(t[:, PHS:PHS + 1], 0.0)
                        nc.gpsimd.memset(t[:, 2 * PHS:2 * PHS + 1], 0.0)
                        # copy both halves out of PSUM
                        nc.vector.tensor_copy(t[:, 1:W + 1], pst[T][:, 0:W])
                        nc.vector.tensor_copy(t[:, PHS + 1:PHS + 1 + W],
                                              pst[T][:, W:2 * W])
                        # zero out-of-image rows
                        for gs in range(3 * T, 3 * T + 3):
                            s = gs % 3
                            for hh in (0, 1):
                                rr = r0 - 1 + 2 * gs + hh
                                if not (0 <= rr < H):
                                    nc.vector.memset(
                                        t[32 * s:32 * s + 32,
                                          hh * PHS + 1:hh * PHS + 1 + W], 0.0)
                    emit_select(sAt, r0)

        # ---------------- Maps (compact [128, CF]) ----------------
        with tc.tile_pool(name="mp", bufs=1) as mp:
            jm = mp.tile([128, CF], F32, tag="jm")
            nc.sync.dma_start(jm[:], io["jmap"][:])
            im = mp.tile([128, CF], F32, tag="im")
            nc.sync.dma_start(im[:], io["imap"][:])
            bm = mp.tile([128, CF], F32, tag="bm")
            nc.sync.dma_start(bm[:], io["basemap"][:])

            def coord_chain(row, base_map, clmax):
                oc = mp.tile([128, CF], F32, tag=f"oc{row}")
                nc.sync.dma_start(
                    oc[:], bass.AP(tensor=ox_dram[:].tensor,
                                   offset=ox_dram[:].offset
                                   + row * (N + 4 * W),
                                   ap=[[CF, 128], [1, CF]]))
                ic = mp.tile([128, CF], F32, tag=f"ic{row}")
                nc.vector.scalar_tensor_tensor(ic[:], oc[:], 2.0, base_map[:],
                                               ALU.mult, ALU.add)
                nc.vector.tensor_scalar(ic[:], ic[:], 0.0, clmax,
                                        ALU.max, ALU.min)
                i32 = mp.tile([128, CF], I32, tag=f"i32{row}")
                nc.vector.tensor_copy(i32[:], ic[:])
                c0f = mp.tile([128, CF], F32, tag=f"c0f{row}")
                nc.vector.tensor_copy(c0f[:], i32[:])
                wf = mp.tile([128, CF], F32, tag=f"wf{row}")
                nc.vector.tensor_tensor(wf[:], ic[:], c0f[:], ALU.subtract)
                msk = mp.tile([128, CF], F32, tag=f"msk{row}")
                nc.vector.tensor_scalar(msk[:], wf[:], 0.0, None, ALU.is_lt)
                nc.vector.tensor_tensor(c0f[:], c0f[:], msk[:], ALU.subtract)
                nc.vector.tensor_tensor(wf[:], ic[:], c0f[:], ALU.subtract)
                # 1 - w on the scalar engine (scale=-1, bias=+1)
                w1m = mp.tile([128, CF], F32, tag=f"w1m{row}")
                nc.scalar.activation(w1m[:], wf[:], AF.Copy, bias=0.0,
                                     scale=-1.0)
                nc.vector.tensor_scalar(w1m[:], w1m[:], 1.0, None, ALU.add)
                return c0f, wf, w1m

            x0f, wx, wx1m = coord_chain(0, jm, CLX)
            y0f, wy, wy1m = coord_chain(1, im, CLY)
            for row, (a, b) in enumerate(
                    ((wx1m, wy1m), (wx, wy1m), (wx1m, wy), (wx, wy))):
                wprod = mp.tile([128, CF], F16, tag=f"wp{row}")
                nc.vector.tensor_tensor(wprod[:], a[:], b[:], ALU.mult)
                nc.sync.dma_start(w4d[row:row + 1, :], wprod[:])
            idxf = mp.tile([128, CF], F32, tag="idxf")
            nc.vector.scalar_tensor_tensor(idxf[:], y0f[:], float(W), x0f[:],
                                           ALU.mult, ALU.add)
            nc.vector.tensor_tensor(idxf[:], idxf[:], bm[:], ALU.subtract)
            idx16 = mp.tile([128, CF], I16, tag="idx16")
            nc.vector.tensor_copy(idx16[:], idxf[:])
            nc.sync.dma_start(idx16d[:], idx16[:])


        # ---------------- Phase B || Phase C ----------------
        with tc.tile_pool(name="w2", bufs=1) as w2p, \
             tc.tile_pool(name="wc", bufs=1) as wc:
            W2 = w2p.tile([128, N + 2 * W], F16, tag="W2")
            wm1_s = wc.tile([C, 3 * C], F16, tag="wm1")
            nc.sync.dma_start(wm1_s[:], io["wm1"][:])
            wm1a_s = wc.tile([C, 3 * C], F16, tag="wm1a")
            nc.sync.dma_start(wm1a_s[:], io["wm1a"][:])
            wm1b_s = wc.tile([C, 3 * C], F16, tag="wm1b")
            nc.sync.dma_start(wm1b_s[:], io["wm1b"][:])
            wm2_s = wc.tile([2 * C, 3 * C], F16, tag="wm2")
            nc.sync.dma_start(wm2_s[:], io["wm2"][:])

            with tc.tile_pool(name="gb", bufs=2) as gbp, \
                 tc.tile_pool(name="wtb", bufs=2) as wtp, \
                 tc.tile_pool(name="ixb", bufs=2) as ixp, \
                 tc.tile_pool(name="plb", bufs=2) as plp, \
                 tc.tile_pool(name="yb", bufs=3) as ybp, \
                 tc.tile_pool(name="psC", bufs=4, space="PSUM") as psC:
                CHK = N // 64  # wrapped cols per chunk tile (8 blocks each)

                def emit_wrap_chunk(c):
                    idxw_s = ixp.tile([16, CHK], I16, tag=f"idxw{c % 2}",
                                      name=f"idxw{c}")
                    nc.sync.dma_start(
                        idxw_s[:],
                        bass.AP(tensor=idx16d[:].tensor,
                                offset=idx16d[:].offset + c * CHK * 16,
                                ap=[[1, 16], [16, CHK]]))
                    for rep in range(8):
                        nc.sync.dma_start(idx16w[c][16 * rep:16 * rep + 16, :],
                                          idxw_s[:])

                emit_wrap_chunk(0)
                for k in range(NBLK + 2):
                    if k < NBLK:
                        if k % 8 == 0 and k // 8 + 1 < 4:
                            emit_wrap_chunk(k // 8 + 1)
                        # ---- Phase B block k: rows [8k, 8k+8) ----
                        r0 = 8 * k
                        base_px = max(0, r0 - 2) * W
                        idxb = ixp.tile([128, MB // 16], I16, tag="idxb")
                        iwt = idx16w[k // 8]
                        nc.sync.dma_start(
                            idxb[:],
                            bass.AP(tensor=iwt[:].tensor,
                                    offset=iwt[:].offset
                                    + (k % 8) * (MB // 16),
                                    ap=[[N // 64, 128], [1, MB // 16]]))
                        g = gbp.tile([128, 2, MB], F16, tag="g")
                        nc.gpsimd.dma_gather(
                            g[:],
                            bass.AP(tensor=x_pm16[:].tensor,
                                    offset=x_pm16[:].offset + base_px * 4 * C,
                                    ap=[[4 * C, N - base_px], [1, 4 * C]]),
                            idxb[:], MB, MB, 4 * C, transpose=True,
                            single_packet=False)
                        w0t = wtp.tile([128, MB], F16, tag="w0t")
                        nc.sync.dma_start(
                            w0t[:], bass.AP(tensor=w4d[:].tensor,
                                            offset=w4d[:].offset + r0 * W,
                                            ap=[[N, 2], [0, C], [1, MB]]))
                        w1t = wtp.tile([128, MB], F16, tag="w1t")
                        nc.sync.dma_start(
                            w1t[:], bass.AP(tensor=w4d[:].tensor,
                                            offset=w4d[:].offset
                                            + 2 * N + r0 * W,
                                            ap=[[N, 2], [0, C], [1, MB]]))
                        g0 = g[:, 0, :]
                        g1 = g[:, 1, :]
                        nc.vector.tensor_tensor(g0, g0, w0t[:], ALU.mult)
                        nc.vector.tensor_tensor(g1, g1, w1t[:], ALU.mult)
                        nc.vector.tensor_tensor(g0, g0, g1, ALU.add)
                        pl = plp.tile([C, MB], F16, tag="pl")
                        nc.scalar.copy(pl[:], g[C:2 * C, 0, :])
                        nc.vector.tensor_tensor(
                            W2[0:C, r0 * W:(r0 + 8) * W], g[0:C, 0, :],
                            pl[:], ALU.add)
                        blo = max(2 * DIL, r0)
                        if blo < r0 + 8:
                            nc.vector.tensor_copy(
                                W2[C:2 * C,
                                   (blo - 2 * DIL) * W:(r0 + 8 - 2 * DIL) * W],
                                W2[0:C, blo * W:(r0 + 8) * W])
                    if k >= 2:
                        # ---- Phase C block k-2: rows [8(k-2), 8(k-2)+8) ----
                        rc0 = 8 * (k - 2)
                        yb = ybp.tile([C, 8 * W], F16, tag="yb")
                        for rr0 in range(rc0, rc0 + 8, 2):
                            ps = psC.tile([C, 2 * W], F32, tag="psC")
                            mms = []
                            for ri in range(2):
                                r = rr0 + ri
                                seg = {-1: (DIL, W, -DIL), 0: (0, W, 0),
                                       1: (0, W - DIL, DIL)}
                                for ds in (0, -1, 1):
                                    olo, ohi, dsoff = seg[ds]
                                    gcol = (ds + 1) * C
                                    pv = ps[:, ri * W + olo:ri * W + ohi]
                                    mms.append(
                                        (pv, wm1_s[:, gcol:gcol + C],
                                         W2[0:C, r * W + olo + dsoff:
                                            r * W + ohi + dsoff]))
                                    if DIL <= r < H - DIL:
                                        mms.append(
                                            (pv, wm2_s[:, gcol:gcol + C],
                                             W2[:, (r - DIL) * W + olo + dsoff:
                                                (r - DIL) * W + ohi + dsoff]))
                                    elif r < DIL:
                                        mms.append(
                                            (pv, wm1b_s[:, gcol:gcol + C],
                                             W2[0:C,
                                                (r + DIL) * W + olo + dsoff:
                                                (r + DIL) * W + ohi + dsoff]))
                                    else:
                                        mms.append(
                                            (pv, wm1a_s[:, gcol:gcol + C],
                                             W2[0:C,
                                                (r - DIL) * W + olo + dsoff:
                                                (r - DIL) * W + ohi + dsoff]))
                            for i, (o, l, rr) in enumerate(mms):
                                nc.tensor.matmul(o, l, rr, start=(i == 0),
                                                 stop=(i == len(mms) - 1))
                            nc.scalar.activation(
                                yb[:, (rr0 - rc0) * W:(rr0 - rc0 + 2) * W],
                                ps[:], AF.Relu, bias=biasy_s[:], scale=1.0)
                        nc.sync.dma_start(y_out[:, rc0 * W:(rc0 + 8) * W],
                                          yb[:])


def build_module(num_devices):
    nc = bacc.Bacc("TRN2", target_bir_lowering=False, debug=False,
                   num_devices=num_devices)
    io = {}
    for name, shape, dt in IN_SPECS:
        mdt = {np.float16: F16, np.float32: F32}[dt]
        io[name] = nc.dram_tensor(name, list(shape), mdt,
                                  kind="ExternalInput").ap()
    io["y"] = nc.dram_tensor("y", [C, N], F16, kind="ExternalOutput").ap()
    with tile.TileContext(nc) as tc:
        emit(tc, io)
    nc.compile()
    return nc


_NC_CACHE = {}


def kernel(x, offset_w, offset_b, conv_w, bn_gamma, bn_beta, bn_mean, bn_var):
    x = np.asarray(x, np.float32)
    offset_w = np.asarray(offset_w, np.float32)
    offset_b = np.asarray(offset_b, np.float32)
    conv_w = np.asarray(conv_w, np.float32)
    bn_gamma = np.asarray(bn_gamma, np.float32)
    bn_beta = np.asarray(bn_beta, np.float32)
    bn_mean = np.asarray(bn_mean, np.float32)
    bn_var = np.asarray(bn_var, np.float32)
    B = x.shape[0]
    if "nc" not in _NC_CACHE:
        _NC_CACHE["nc"] = build_module(N_CORES)
    nc = _NC_CACHE["nc"]
    shared = prep_shared(offset_w, offset_b, conv_w, bn_gamma, bn_beta,
                         bn_mean, bn_var)
    in_maps = []
    for b in range(B):
        m = dict(shared)
        m.update(prep_sample(x[b]))
        in_maps.append(m)
    res = bass_utils.run_bass_kernel_spmd(nc, in_maps,
                                          core_ids=list(range(B)))
    out = np.stack([res.results[b]["y"].reshape(C, H, W) for b in range(B)])
    return out.astype(np.float32)
